# revision 42
# baseline (speedup 1.0000x reference)
"""Trainium2 Bass kernel for nn_Attention (B=2, C=256, H=W=64, 8 heads).

Sharding: 8 cores = 2 batches x 4 query-chunks (1024 queries each), no
collectives. Each core gets its batch's full x (bf16) with token columns
rolled so its own query chunk sits at columns 0:1024 (attention is
permutation-invariant over keys); it computes LN + projections + attention
for its queries and writes a [256, 1024] slice of the output.

Key structure (v2 — rebuilt for PE throughput; 703us -> 470us):
- x ships as bf16 (attention path + LN stats) plus an f32 query-slice for
  the exact residual.
- LN is folded into the projections: gamma is pre-multiplied into the
  weights on the host; the per-token mean and sqrt(var+eps) enter as two
  rank-1 fixup matmuls accumulated into each projection's PSUM
  (-rowsum(W') x mu and (W@beta) x srt), followed by a *rstd multiply at
  PSUM evacuation. No normalized-x tensor is ever materialized. rstd and
  sqrt come from bf16 exponent bit-tricks on VectorE (the custom-DVE fast
  reciprocal is broken on this stack, and mixing Ln into ScalarE thrashes
  the ACT table against Exp at 2.7us/reload).
- S^T matmuls are full-array (unmasked): the stationary is the whole
  4-head K chunk [128x128]; per-head Q lives in zero-padded [128, NQ]
  tiles so each 512-query matmul contracts over all 128 channel rows but
  only the head's 32 rows are nonzero. This keeps the PE HAM clock-gate
  at 2.4 GHz (masked tile_position matmuls don't count as PE activity —
  the whole attention phase otherwise runs at the cold 1.2 GHz clock).
- exp splits per key-chunk between ScalarE (true exp on head-pair 0) and
  VectorE (Schraudolph bit-trick on head-pair 1): Wq is pre-scaled so
  PSUM = 128*log2(e)*logit; adding a magic bias and converting f32->int16
  yields the bf16 bit pattern of 2^y (~3% max rel err, harmless next to
  the residual). Both pairs process [128,1024] two-bank PSUM tiles.
- P^T V via stationary-V matmuls whose 33rd per-head column is a dummy
  output channel (zero weights, rank-1 bias = srt, so after the *rstd
  evacuation it is exactly 1): the AV accumulation emits the softmax
  denominators for free. Normalization is a bf16 bit-trick reciprocal
  broadcast down 32 rows through a tiny bf16 matmul.
- Per-f output projection (+bias +f32 residual in one fused
  scalar_tensor_tensor) overlaps the next attention group.
"""

import numpy as np

B, C, H, W = 2, 256, 64, 64
N = H * W            # 4096 tokens
NH, HD = 8, 32       # heads, head_dim
NQ = N // 4          # queries per core
LN_EPS = 1e-5
LOG2E = 1.4426950408889634
LN2 = 0.6931471805599453
ATTN_SCALE = HD ** -0.5
A_SCALE = 128.0 * LOG2E * ATTN_SCALE   # folded into Wq on host
B16F = 16256.0 - 5.6                   # Schraudolph bias (calibrated)
KRSQ = 24375.25                        # bf16 bit-trick rsqrt bias (<=3.7% rel)
KSQ = 8123.0                           # bf16 bit-trick sqrt bias (<=3.9% rel)
KRCP = 32498.75                        # bf16 bit-trick recip bias (<=5.3% rel)

_PROFILE = False
_CACHE = {}


def _build():
    from concourse import bacc
    from concourse import mybir
    import concourse.tile as tile

    f32 = mybir.dt.float32
    f32r = mybir.dt.float32r
    bf16 = mybir.dt.bfloat16
    i16 = mybir.dt.int16
    ALU = mybir.AluOpType
    ACTF = mybir.ActivationFunctionType

    nc = bacc.Bacc("TRN2", target_bir_lowering=False)
    xbd = nc.dram_tensor("xb", [C, N], bf16, kind="ExternalInput")
    xfd = nc.dram_tensor("xf", [C, NQ], f32, kind="ExternalInput")
    wq = nc.dram_tensor("wqT", [C, C], bf16, kind="ExternalInput")  # gamma+A_SCALE folded
    wk = nc.dram_tensor("wkT", [C, C], bf16, kind="ExternalInput")
    wv = nc.dram_tensor("wvT", [C, NH * 33], bf16, kind="ExternalInput")
    wp = nc.dram_tensor("wpT", [C, C], bf16, kind="ExternalInput")
    # rank-1 LN fixup rows: *0 = -rowsum(W'), *1 = W@beta (each own tensor so
    # every engine/matmul access starts at partition 0)
    wbq0 = nc.dram_tensor("wbq0", [1, C], bf16, kind="ExternalInput")
    wbq1 = nc.dram_tensor("wbq1", [1, C], bf16, kind="ExternalInput")
    wbk0 = nc.dram_tensor("wbk0", [1, C], bf16, kind="ExternalInput")
    wbk1 = nc.dram_tensor("wbk1", [1, C], bf16, kind="ExternalInput")
    wbv0 = nc.dram_tensor("wbv0", [1, NH * 33], bf16, kind="ExternalInput")
    wbv1 = nc.dram_tensor("wbv1", [1, NH * 33], bf16, kind="ExternalInput")
    bpd = nc.dram_tensor("bp", [C, 1], f32, kind="ExternalInput")
    od = nc.dram_tensor("out", [C, NQ], f32, kind="ExternalOutput")

    with tile.TileContext(nc) as tc:
        with tc.tile_pool(name="big", bufs=1) as big, \
             tc.tile_pool(name="sml", bufs=2) as sml:

            # ---- load inputs ----
            xb = [big.tile([128, N], bf16, tag=f"xb{c}", name=f"xb{c}") for c in range(2)]
            for q4 in range(4):
                qs = slice(q4 * 1024, (q4 + 1) * 1024)
                for c in range(2):
                    nc.sync.dma_start(out=xb[c][:, qs], in_=xbd[c * 128:(c + 1) * 128, qs])
            xf = [big.tile([128, NQ], f32, tag=f"xf{c}", name=f"xf{c}") for c in range(2)]
            for c in range(2):
                nc.sync.dma_start(out=xf[c][:, :], in_=xfd[c * 128:(c + 1) * 128, :])
            w_sb = {}
            for name, t, nout in (("q", wq, C), ("k", wk, C),
                                  ("v", wv, NH * 33), ("p", wp, C)):
                for c in range(2):
                    s = big.tile([128, nout], bf16, tag=f"w{name}{c}", name=f"w{name}{c}")
                    nc.sync.dma_start(out=s[:, :], in_=t[c * 128:(c + 1) * 128, :])
                    w_sb[name, c] = s
            wb_sb = {}
            for name, t, nout in (("q0", wbq0, C), ("q1", wbq1, C),
                                  ("k0", wbk0, C), ("k1", wbk1, C),
                                  ("v0", wbv0, NH * 33), ("v1", wbv1, NH * 33)):
                s = big.tile([1, nout], bf16, tag=f"wb{name}", name=f"wb{name}")
                nc.sync.dma_start(out=s[:, :], in_=t[:, :])
                wb_sb[name] = s
            bp_sb = [big.tile([128, 1], f32, tag=f"bp{c}", name=f"bp{c}") for c in range(2)]
            for c in range(2):
                nc.sync.dma_start(out=bp_sb[c][:, :], in_=bpd[c * 128:(c + 1) * 128, :])

            onesC = big.tile([128, 1], bf16, tag="onesC", name="onesC")
            nc.vector.memset(onesC[:, :], 1.0 / C)
            ones_row = big.tile([1, 128], bf16, tag="onesr", name="onesr")
            nc.vector.memset(ones_row[:, :], 1.0)
            ident = big.tile([1, 1], f32, tag="ident", name="ident")
            nc.vector.memset(ident[:, :], 1.0)

            rs_cols = big.tile([128, 32], f32, tag="rscols", name="rscols")  # rstd, col layout

            kT = [big.tile([128, N], bf16, tag=f"kT{c}", name=f"kT{c}") for c in range(2)]
            qp = [[big.tile([128, NQ], bf16, tag=f"qp{hg}{h}", name=f"qp{hg}{h}")
                   for h in range(4)] for hg in range(2)]
            for hg in range(2):
                for h in range(4):
                    nc.gpsimd.memset(qp[hg][h][:, :], 0.0)
            v_sb = big.tile([128, 32, NH, 33], bf16, tag="v", name="v")
            attnT = [big.tile([128, NQ], bf16, tag=f"at{c}", name=f"at{c}") for c in range(2)]

            # ---- LN stats + projections ----
            with tc.tile_pool(name="lnsb", bufs=1) as lnsb, \
                 tc.tile_pool(name="lnp", bufs=1, space="PSUM") as lnp, \
                 tc.tile_pool(name="wrm", bufs=2, space="PSUM") as wrm, \
                 tc.tile_pool(name="mm", bufs=2, space="PSUM") as mmp:
                # ~4.4us of back-to-back full-array matmuls on the (tiny,
                # early-arriving) weight tiles: releases the HAM clock-gate to
                # 2.4 GHz during the x DMA wait, so the stats/fixup phase (all
                # masked matmuls that don't count as PE activity) runs warm.
                for w in range(20):
                    wps = wrm.tile([128, NH * 33], f32, tag="w", name="w")
                    nc.tensor.matmul(wps[:, :], w_sb["p", w % 2][:, 0:128],
                                     w_sb["v", w % 2][:, :], start=True, stop=True)
                mu_row = lnsb.tile([1, N], bf16, tag="murow", name="murow")
                srt_row = lnsb.tile([1, N], bf16, tag="srtrow", name="srtrow")
                rs_row = lnsb.tile([1, N], f32, tag="rsrow", name="rsrow")
                rs_bf = lnsb.tile([1, N], bf16, tag="rsbf", name="rsbf")
                rs_ball = lnsb.tile([128, N], f32, tag="rsball", name="rsball")
                xsq = [lnsb.tile([128, N], bf16, tag=f"xsq{c}", name=f"xsq{c}") for c in range(2)]
                nc.vector.tensor_tensor(xsq[0][:, :], xb[0][:, :], xb[0][:, :], ALU.mult)
                nc.vector.tensor_tensor(xsq[1][:, :], xb[1][:, :], xb[1][:, :], ALU.mult)
                rsT_ps = lnp.tile([128, 32], f32, tag="rsT", name="rsT")
                for f in range(8):
                    fl = slice(f * 512, (f + 1) * 512)
                    mps = lnp.tile([1, 512], f32, tag="mps", name="mps")
                    nc.tensor.matmul(mps[:, :], onesC[:, :], xb[0][:, fl], start=True, stop=False)
                    nc.tensor.matmul(mps[:, :], onesC[:, :], xb[1][:, fl], start=False, stop=True)
                    sps = lnp.tile([1, 512], f32, tag="sps", name="sps")
                    nc.tensor.matmul(sps[:, :], onesC[:, :], xsq[0][:, fl], start=True, stop=False)
                    nc.tensor.matmul(sps[:, :], onesC[:, :], xsq[1][:, fl], start=False, stop=True)
                    # mu row (SBUF, bf16) + vare = (msq + eps) - mu^2
                    nc.scalar.copy(mu_row[0:1, fl], mps[:, :])
                    mu2 = sml.tile([1, 512], f32, tag="mu2", name="mu2")
                    nc.vector.tensor_tensor(mu2[:, :], mu_row[0:1, fl], mu_row[0:1, fl], ALU.mult)
                    vare = sml.tile([1, 512], f32, tag="vare", name="vare")
                    nc.vector.scalar_tensor_tensor(vare[:, :], sps[:, :], LN_EPS, mu2[:, :],
                                                   ALU.add, ALU.subtract)
                    # rstd / sqrt via bf16 exponent bit tricks on VectorE
                    # (keeps ScalarE's ACT table pinned to the Exp set; the
                    # custom-DVE fast reciprocal is broken on this stack)
                    vb = sml.tile([1, 512], bf16, tag="vb", name="vb")
                    nc.vector.tensor_copy(vb[:, :], vare[:, :])
                    nc.vector.tensor_scalar(rs_bf[0:1, fl].bitcast(i16),
                                            vb[:, :].bitcast(i16),
                                            -0.5, KRSQ, ALU.mult, ALU.add)
                    nc.vector.tensor_scalar(srt_row[0:1, fl].bitcast(i16),
                                            vb[:, :].bitcast(i16),
                                            0.5, KSQ, ALU.mult, ALU.add)
                    nc.scalar.copy(rs_row[0:1, fl], rs_bf[0:1, fl])
                    # rs broadcast down 128 rows (bf16 matmul) -> SBUF
                    rsb_ps = lnp.tile([128, 512], f32, tag="rsb", name="rsb")
                    nc.tensor.matmul(rsb_ps[:, :], ones_row[:, :], rs_bf[0:1, fl],
                                     start=True, stop=True)
                    nc.vector.tensor_copy(rs_ball[:, fl], rsb_ps[:, :])
                    # rstd row -> column layout (PE transposes, 4 chunks of 128)
                    for t in range(4):
                        j = f * 4 + t
                        nc.tensor.transpose(rsT_ps[:, j:j + 1],
                                            rs_row[0:1, j * 128:(j + 1) * 128], ident[:, :])
                    nc.vector.tensor_copy(rs_cols[:, f * 4:f * 4 + 4],
                                          rsT_ps[:, f * 4:f * 4 + 4])

                    # K projection for this token chunk (both output halves)
                    for co in range(2):
                        cs = slice(co * 128, (co + 1) * 128)
                        ps = mmp.tile([128, 512], f32, tag="proj", name="proj")
                        for ci in range(2):
                            nc.tensor.matmul(ps[:, :], w_sb["k", ci][:, cs],
                                             xb[ci][:, fl], start=(ci == 0), stop=False)
                        nc.tensor.matmul(ps[:, :], wb_sb["k0"][:, cs],
                                         mu_row[0:1, fl], start=False, stop=False)
                        nc.tensor.matmul(ps[:, :], wb_sb["k1"][:, cs],
                                         srt_row[0:1, fl], start=False, stop=True)
                        nc.vector.tensor_tensor(kT[co][:, fl], ps[:, :], rs_ball[:, fl], ALU.mult)

                    # Q projection (only first two chunks = this core's queries)
                    if f < 2:
                        for co in range(2):
                            cs = slice(co * 128, (co + 1) * 128)
                            ps = mmp.tile([128, 512], f32, tag="proj", name="proj")
                            for ci in range(2):
                                nc.tensor.matmul(ps[:, :], w_sb["q", ci][:, cs],
                                                 xb[ci][:, fl], start=(ci == 0), stop=False)
                            nc.tensor.matmul(ps[:, :], wb_sb["q0"][:, cs],
                                             mu_row[0:1, fl], start=False, stop=False)
                            nc.tensor.matmul(ps[:, :], wb_sb["q1"][:, cs],
                                             srt_row[0:1, fl], start=False, stop=True)
                            for h in range(4):
                                rr = slice(h * 32, (h + 1) * 32)
                                nc.vector.tensor_tensor(qp[co][h][rr, fl], ps[rr, :],
                                                        rs_ball[rr, fl], ALU.mult)

                # V projection per 128-token chunk (tokens in partitions). The
                # 33rd "dummy" channel per head has zero weights and rank-1
                # bias = sqrt(var+eps), so after the *rstd evacuation it is
                # exactly the ones column (softmax denominator accumulator).
                for j in range(32):
                    jl = slice(j * 128, (j + 1) * 128)
                    ps = mmp.tile([128, NH * 33], f32, tag="proj", name="vproj")
                    for ci in range(2):
                        nc.tensor.matmul(ps[:, :], xb[ci][:, jl], w_sb["v", ci][:, :],
                                         start=(ci == 0), stop=False)
                    nc.tensor.matmul(ps[:, :], mu_row[0:1, jl],
                                     wb_sb["v0"][:, :], start=False, stop=False)
                    nc.tensor.matmul(ps[:, :], srt_row[0:1, jl],
                                     wb_sb["v1"][:, :], start=False, stop=True)
                    nc.scalar.mul(v_sb[:, j, :, :],
                                  ps[:, :].rearrange("p (h e) -> p h e", h=NH),
                                  rs_cols[:, j:j + 1])

            # ---- attention ----
            with tc.tile_pool(name="sps", bufs=1, space="PSUM") as sp, \
                 tc.tile_pool(name="avp", bufs=1, space="PSUM") as avp, \
                 tc.tile_pool(name="bcp", bufs=1, space="PSUM") as bcp, \
                 tc.tile_pool(name="pp", bufs=2) as ppool, \
                 tc.tile_pool(name="nrm", bufs=2) as nrm:
                with tc.tile_pool(name="mm2", bufs=1, space="PSUM") as mm2, \
                     tc.tile_pool(name="ot", bufs=4) as otp:
                    for f in range(2):
                        fl = slice(f * 512, (f + 1) * 512)
                        for hg in range(2):
                            av = [avp.tile([128, 512], f32, tag=f"av{pr}", name=f"av{pr}")
                                  for pr in range(2)]
                            for j in range(32):
                                jl = slice(j * 128, (j + 1) * 128)
                                ss = [sp.tile([128, 1024], f32, tag=f"s{i}", name=f"s{i}")
                                      for i in range(2)]
                                pt = [ppool.tile([128, 1024], bf16, tag=f"p{i}", name=f"p{i}")
                                      for i in range(2)]
                                for i in range(2):
                                    for t2 in range(2):
                                        h = i * 2 + t2
                                        nc.tensor.matmul(ss[i][:, t2 * 512:(t2 + 1) * 512],
                                                         kT[hg][:, jl], qp[hg][h][:, fl],
                                                         start=True, stop=True)
                                # pair 0: true exp on ScalarE; pair 1: Schraudolph on VectorE
                                nc.scalar.activation(pt[0][:, :], ss[0][:, :],
                                                     ACTF.Exp, scale=LN2 / 128.0)
                                nc.vector.tensor_scalar(pt[1][:, :].bitcast(i16), ss[1][:, :],
                                                        B16F, None, ALU.add)
                                for pr in range(2):
                                    for t2 in range(2):
                                        h = pr * 2 + t2
                                        nc.tensor.matmul(
                                            av[pr][t2 * 64:t2 * 64 + 33, :],
                                            v_sb[:, j, hg * 4 + h, :],
                                            pt[pr][:, t2 * 512:(t2 + 1) * 512],
                                            start=(j == 0), stop=(j == 31),
                                            tile_position=(0, t2 * 64))
                            # normalization: bit-trick reciprocal of the PSUM
                            # ones-row denominator, broadcast, multiply
                            for pr in range(2):
                                for t2 in range(2):
                                    db = nrm.tile([1, 512], bf16, tag="db", name="db")
                                    nc.vector.tensor_copy(
                                        db[:, :], av[pr][t2 * 64 + 32:t2 * 64 + 33, :])
                                    rcpb = nrm.tile([1, 512], bf16, tag="rb", name="rb")
                                    nc.vector.tensor_scalar(rcpb[:, :].bitcast(i16),
                                                            db[:, :].bitcast(i16),
                                                            -1.0, KRCP, ALU.mult, ALU.add)
                                    bcq = bcp.tile([32, 512], f32, tag="bcq", name="bcq")
                                    nc.tensor.matmul(bcq[:, :], ones_row[:, 0:32],
                                                     rcpb[:, :], start=True, stop=True)
                                    bcs = nrm.tile([32, 512], bf16, tag="bcs", name="bcs")
                                    nc.vector.tensor_copy(bcs[:, :], bcq[:, :])
                                    row0 = (pr * 2 + t2) * 32
                                    nc.vector.tensor_tensor(
                                        attnT[hg][row0:row0 + 32, fl],
                                        av[pr][t2 * 64:t2 * 64 + 32, :],
                                        bcs[:, :], ALU.mult)
                        # output projection + bias + residual for this f-chunk
                        # (overlaps the next attention group's matmuls)
                        for mo in range(2):
                            ms = slice(mo * 128, (mo + 1) * 128)
                            ps = mm2.tile([128, 512], f32, tag="o", name="o")
                            for ci in range(2):
                                nc.tensor.matmul(ps[:, :], w_sb["p", ci][:, ms],
                                                 attnT[ci][:, fl], start=(ci == 0), stop=(ci == 1))
                            ot = otp.tile([128, 512], f32, tag="ot", name="ot")
                            nc.vector.scalar_tensor_tensor(ot[:, :], ps[:, :], bp_sb[mo][:, :],
                                                           xf[mo][:, fl], ALU.add, ALU.add)
                            nc.sync.dma_start(out=od[ms, fl], in_=ot[:, :])

    nc.finalize()
    return nc


def kernel(x, ln_gamma, ln_beta, w_qkv, w_proj, b_proj):
    import ml_dtypes
    from concourse.bass_utils import run_bass_kernel_spmd

    if "nc" not in _CACHE:
        _CACHE["nc"] = _build()
    nc = _CACHE["nc"]

    bf = ml_dtypes.bfloat16
    x = np.asarray(x, np.float32)
    w_qkv = np.asarray(w_qkv, np.float32)
    gam = np.asarray(ln_gamma, np.float32)
    bet = np.asarray(ln_beta, np.float32)
    wq_, wk_, wv_ = w_qkv[0:C], w_qkv[C:2 * C], w_qkv[2 * C:3 * C]

    def prep(wmat, scale):
        wg = (scale * wmat * gam[None, :]).astype(bf)           # [o, c] gamma folded
        wT = np.ascontiguousarray(wg.T)                         # lhsT layout [in, out]
        sw = wg.astype(np.float32).sum(1)                       # rowsum of device weights
        bias = scale * (wmat @ bet)
        return (wT, np.ascontiguousarray(-sw[None, :].astype(bf)),
                np.ascontiguousarray(bias[None, :].astype(bf)))

    wqT, wbq0_h, wbq1_h = prep(wq_, A_SCALE)
    wkT, wbk0_h, wbk1_h = prep(wk_, 1.0)
    # V extended with a zero-weight dummy channel per head whose rank-1 bias
    # is 1 against the srt row (becomes the softmax-denominator ones column).
    wvg = (wv_ * gam[None, :]).astype(bf)
    wv_ext = np.zeros((NH * 33, C), bf)
    wbv0_h = np.zeros((1, NH * 33), np.float32)
    wbv1_h = np.zeros((1, NH * 33), np.float32)
    for h in range(NH):
        wv_ext[h * 33:h * 33 + 32] = wvg[h * 32:(h + 1) * 32]
        wbv0_h[0, h * 33:h * 33 + 32] = -wvg[h * 32:(h + 1) * 32].astype(np.float32).sum(1)
        wbv1_h[0, h * 33:h * 33 + 32] = (wv_ @ bet)[h * 32:(h + 1) * 32]
        wbv1_h[0, h * 33 + 32] = 1.0
    wvT = np.ascontiguousarray(wv_ext.T)
    wbv0_h = wbv0_h.astype(bf)
    wbv1_h = wbv1_h.astype(bf)
    wpT = np.ascontiguousarray(np.asarray(w_proj, np.float32).T.astype(bf))
    bp = np.asarray(b_proj, np.float32).reshape(C, 1)

    xfull = x.reshape(B, C, N)
    in_maps = []
    for core in range(8):
        b, qc = core // 4, core % 4
        xr = np.roll(xfull[b], -qc * NQ, axis=1)
        in_maps.append({
            "xb": np.ascontiguousarray(xr.astype(bf)),
            "xf": np.ascontiguousarray(xr[:, :NQ]),
            "wqT": wqT, "wkT": wkT, "wvT": wvT, "wpT": wpT,
            "wbq0": wbq0_h, "wbq1": wbq1_h, "wbk0": wbk0_h, "wbk1": wbk1_h,
            "wbv0": wbv0_h, "wbv1": wbv1_h, "bp": bp,
        })

    res = run_bass_kernel_spmd(nc, in_maps, core_ids=list(range(8)),
                               trace=_PROFILE)
    if _PROFILE:
        _CACHE["exec_time_ns"] = res.exec_time_ns
    out = np.empty((B, C, N), np.float32)
    for core in range(8):
        b, qc = core // 4, core % 4
        out[b][:, qc * NQ:(qc + 1) * NQ] = res.results[core]["out"]
    return out.reshape(B, C, H, W)


# revision 43
# speedup vs baseline: 1.0334x; 1.0334x over previous
"""Trainium2 Bass kernel for nn_Attention (B=2, C=256, H=W=64, 8 heads).

Sharding: 8 cores = 2 batches x 4 query-chunks (1024 queries each), no
collectives. Each core gets its batch's full x (bf16) with token columns
rolled so its own query chunk sits at columns 0:1024 (attention is
permutation-invariant over keys); it computes LN + projections + attention
for its queries and writes a [256, 1024] slice of the output.

Key structure (v2 — rebuilt for PE throughput; 703us -> 470us):
- x ships as bf16 (attention path + LN stats) plus an f32 query-slice for
  the exact residual.
- LN is folded into the projections: gamma is pre-multiplied into the
  weights on the host; the per-token mean and sqrt(var+eps) enter as two
  rank-1 fixup matmuls accumulated into each projection's PSUM
  (-rowsum(W') x mu and (W@beta) x srt), followed by a *rstd multiply at
  PSUM evacuation. No normalized-x tensor is ever materialized. rstd and
  sqrt come from bf16 exponent bit-tricks on VectorE (the custom-DVE fast
  reciprocal is broken on this stack, and mixing Ln into ScalarE thrashes
  the ACT table against Exp at 2.7us/reload).
- S^T matmuls are full-array (unmasked): the stationary is the whole
  4-head K chunk [128x128]; per-head Q lives in zero-padded [128, NQ]
  tiles so each 512-query matmul contracts over all 128 channel rows but
  only the head's 32 rows are nonzero. This keeps the PE HAM clock-gate
  at 2.4 GHz (masked tile_position matmuls don't count as PE activity —
  the whole attention phase otherwise runs at the cold 1.2 GHz clock).
- exp splits per key-chunk between ScalarE (true exp on head-pair 0) and
  VectorE (Schraudolph bit-trick on head-pair 1): Wq is pre-scaled so
  PSUM = 128*log2(e)*logit; adding a magic bias and converting f32->int16
  yields the bf16 bit pattern of 2^y (~3% max rel err, harmless next to
  the residual). Both pairs process [128,1024] two-bank PSUM tiles.
- P^T V via stationary-V matmuls whose 33rd per-head column is a dummy
  output channel (zero weights, rank-1 bias = srt, so after the *rstd
  evacuation it is exactly 1): the AV accumulation emits the softmax
  denominators for free. Normalization is a bf16 bit-trick reciprocal
  broadcast down 32 rows through a tiny bf16 matmul.
- Per-f output projection (+bias +f32 residual in one fused
  scalar_tensor_tensor) overlaps the next attention group.
"""

import numpy as np

B, C, H, W = 2, 256, 64, 64
N = H * W            # 4096 tokens
NH, HD = 8, 32       # heads, head_dim
NQ = N // 4          # queries per core
LN_EPS = 1e-5
LOG2E = 1.4426950408889634
LN2 = 0.6931471805599453
ATTN_SCALE = HD ** -0.5
A_SCALE = 128.0 * LOG2E * ATTN_SCALE   # folded into Wq on host
B16F = 16256.0 - 5.6                   # Schraudolph bias (calibrated)
KRSQ = 24375.25                        # bf16 bit-trick rsqrt bias (<=3.7% rel)
KSQ = 8123.0                           # bf16 bit-trick sqrt bias (<=3.9% rel)
KRCP = 32498.75                        # bf16 bit-trick recip bias (<=5.3% rel)

_PROFILE = False
_CACHE = {}


def _build():
    from concourse import bacc
    from concourse import mybir
    import concourse.tile as tile

    f32 = mybir.dt.float32
    f32r = mybir.dt.float32r
    bf16 = mybir.dt.bfloat16
    i16 = mybir.dt.int16
    ALU = mybir.AluOpType
    ACTF = mybir.ActivationFunctionType

    nc = bacc.Bacc("TRN2", target_bir_lowering=False)
    xbd = nc.dram_tensor("xb", [C, N], bf16, kind="ExternalInput")
    xfd = nc.dram_tensor("xf", [C, NQ], f32, kind="ExternalInput")
    wq = nc.dram_tensor("wqT", [C, C], bf16, kind="ExternalInput")  # gamma+A_SCALE folded
    wk = nc.dram_tensor("wkT", [C, C], bf16, kind="ExternalInput")
    wv = nc.dram_tensor("wvT", [C, NH * 33], bf16, kind="ExternalInput")
    wp = nc.dram_tensor("wpT", [C, C], bf16, kind="ExternalInput")
    # rank-1 LN fixup pairs: row0 = W@beta (pairs sqrt(var+eps)), row1 =
    # -rowsum(W') (pairs mu) — one contract-2 fixup matmul per projection chunk
    wbq = nc.dram_tensor("wbq", [2, C], bf16, kind="ExternalInput")
    wbk = nc.dram_tensor("wbk", [2, C], bf16, kind="ExternalInput")
    wbv = nc.dram_tensor("wbv", [2, NH * 33], bf16, kind="ExternalInput")
    bpd = nc.dram_tensor("bp", [C, 1], f32, kind="ExternalInput")
    od = nc.dram_tensor("out", [C, NQ], f32, kind="ExternalOutput")

    with tile.TileContext(nc) as tc:
        with tc.tile_pool(name="big", bufs=1) as big, \
             tc.tile_pool(name="sml", bufs=2) as sml:

            # ---- load inputs ----
            xb = [big.tile([128, N], bf16, tag=f"xb{c}", name=f"xb{c}") for c in range(2)]
            for q4 in range(4):
                qs = slice(q4 * 1024, (q4 + 1) * 1024)
                for c in range(2):
                    nc.sync.dma_start(out=xb[c][:, qs], in_=xbd[c * 128:(c + 1) * 128, qs])
            xf = [big.tile([128, NQ], f32, tag=f"xf{c}", name=f"xf{c}") for c in range(2)]
            for c in range(2):
                nc.sync.dma_start(out=xf[c][:, :], in_=xfd[c * 128:(c + 1) * 128, :])
            w_sb = {}
            for name, t, nout in (("q", wq, C), ("k", wk, C),
                                  ("v", wv, NH * 33), ("p", wp, C)):
                for c in range(2):
                    s = big.tile([128, nout], bf16, tag=f"w{name}{c}", name=f"w{name}{c}")
                    nc.sync.dma_start(out=s[:, :], in_=t[c * 128:(c + 1) * 128, :])
                    w_sb[name, c] = s
            wb_sb = {}
            for name, t, nout in (("q", wbq, C), ("k", wbk, C), ("v", wbv, NH * 33)):
                s = big.tile([2, nout], bf16, tag=f"wb{name}", name=f"wb{name}")
                nc.sync.dma_start(out=s[:, :], in_=t[:, :])
                wb_sb[name] = s
            bp_sb = [big.tile([128, 1], f32, tag=f"bp{c}", name=f"bp{c}") for c in range(2)]
            for c in range(2):
                nc.sync.dma_start(out=bp_sb[c][:, :], in_=bpd[c * 128:(c + 1) * 128, :])

            onesC = big.tile([128, 1], bf16, tag="onesC", name="onesC")
            nc.vector.memset(onesC[:, :], 1.0 / C)
            ones_row = big.tile([1, 128], bf16, tag="onesr", name="onesr")
            nc.vector.memset(ones_row[:, :], 1.0)
            ident = big.tile([1, 1], f32, tag="ident", name="ident")
            nc.vector.memset(ident[:, :], 1.0)

            rs_cols = big.tile([128, 32], f32, tag="rscols", name="rscols")  # rstd, col layout

            kT = [big.tile([128, N], bf16, tag=f"kT{c}", name=f"kT{c}") for c in range(2)]
            qp = [[big.tile([128, NQ], bf16, tag=f"qp{hg}{h}", name=f"qp{hg}{h}")
                   for h in range(4)] for hg in range(2)]
            for hg in range(2):
                for h in range(4):
                    nc.gpsimd.memset(qp[hg][h][:, :], 0.0)
            v_sb = big.tile([128, 32, NH, 33], bf16, tag="v", name="v")
            attnT = [big.tile([128, NQ], bf16, tag=f"at{c}", name=f"at{c}") for c in range(2)]

            # ---- LN stats + projections ----
            with tc.tile_pool(name="lnsb", bufs=1) as lnsb, \
                 tc.tile_pool(name="lnp", bufs=1, space="PSUM") as lnp, \
                 tc.tile_pool(name="wrm", bufs=2, space="PSUM") as wrm, \
                 tc.tile_pool(name="mm", bufs=2, space="PSUM") as mmp:
                # ~4.4us of back-to-back full-array matmuls on the (tiny,
                # early-arriving) weight tiles: releases the HAM clock-gate to
                # 2.4 GHz during the x DMA wait, so the stats/fixup phase (all
                # masked matmuls that don't count as PE activity) runs warm.
                for w in range(20):
                    wps = wrm.tile([128, NH * 33], f32, tag="w", name="w")
                    nc.tensor.matmul(wps[:, :], w_sb["p", w % 2][:, 0:128],
                                     w_sb["v", w % 2][:, :], start=True, stop=True)
                mu_row = lnsb.tile([1, N], bf16, tag="murow", name="murow")
                musrt = lnsb.tile([2, N], bf16, tag="musrt", name="musrt")
                rs_row = lnsb.tile([1, N], f32, tag="rsrow", name="rsrow")
                rs_bf = lnsb.tile([1, N], bf16, tag="rsbf", name="rsbf")
                rs_ball = lnsb.tile([128, N], f32, tag="rsball", name="rsball")
                xsq = [lnsb.tile([128, N], bf16, tag=f"xsq{c}", name=f"xsq{c}") for c in range(2)]
                nc.vector.tensor_tensor(xsq[0][:, :], xb[0][:, :], xb[0][:, :], ALU.mult)
                nc.vector.tensor_tensor(xsq[1][:, :], xb[1][:, :], xb[1][:, :], ALU.mult)
                rsT_ps = lnp.tile([128, 32], f32, tag="rsT", name="rsT")
                for f in range(8):
                    fl = slice(f * 512, (f + 1) * 512)
                    mps = lnp.tile([1, 512], f32, tag="mps", name="mps")
                    nc.tensor.matmul(mps[:, :], onesC[:, :], xb[0][:, fl], start=True, stop=False)
                    nc.tensor.matmul(mps[:, :], onesC[:, :], xb[1][:, fl], start=False, stop=True)
                    sps = lnp.tile([1, 512], f32, tag="sps", name="sps")
                    nc.tensor.matmul(sps[:, :], onesC[:, :], xsq[0][:, fl], start=True, stop=False)
                    nc.tensor.matmul(sps[:, :], onesC[:, :], xsq[1][:, fl], start=False, stop=True)
                    # mu row (SBUF, bf16) + vare = (msq + eps) - mu^2
                    nc.scalar.copy(mu_row[0:1, fl], mps[:, :])
                    mu2 = sml.tile([1, 512], f32, tag="mu2", name="mu2")
                    nc.vector.tensor_tensor(mu2[:, :], mu_row[0:1, fl], mu_row[0:1, fl], ALU.mult)
                    vare = sml.tile([1, 512], f32, tag="vare", name="vare")
                    nc.vector.scalar_tensor_tensor(vare[:, :], sps[:, :], LN_EPS, mu2[:, :],
                                                   ALU.add, ALU.subtract)
                    # rstd / sqrt via bf16 exponent bit tricks on VectorE
                    # (keeps ScalarE's ACT table pinned to the Exp set; the
                    # custom-DVE fast reciprocal is broken on this stack)
                    vb = sml.tile([1, 512], bf16, tag="vb", name="vb")
                    nc.vector.tensor_copy(vb[:, :], vare[:, :])
                    nc.vector.tensor_scalar(rs_bf[0:1, fl].bitcast(i16),
                                            vb[:, :].bitcast(i16),
                                            -0.5, KRSQ, ALU.mult, ALU.add)
                    nc.vector.tensor_scalar(musrt[0:1, fl].bitcast(i16),
                                            vb[:, :].bitcast(i16),
                                            0.5, KSQ, ALU.mult, ALU.add)
                    nc.scalar.copy(rs_row[0:1, fl], rs_bf[0:1, fl])
                    # mu into musrt row1: engines cannot write partition 1,
                    # but an SBUF->SBUF DMA can
                    nc.sync.dma_start(out=musrt[1:2, fl], in_=mu_row[0:1, fl])
                    # rs broadcast down 128 rows (bf16 matmul) -> SBUF
                    rsb_ps = lnp.tile([128, 512], f32, tag="rsb", name="rsb")
                    nc.tensor.matmul(rsb_ps[:, :], ones_row[:, :], rs_bf[0:1, fl],
                                     start=True, stop=True)
                    nc.vector.tensor_copy(rs_ball[:, fl], rsb_ps[:, :])
                    # rstd row -> column layout (PE transposes, 4 chunks of 128)
                    for t in range(4):
                        j = f * 4 + t
                        nc.tensor.transpose(rsT_ps[:, j:j + 1],
                                            rs_row[0:1, j * 128:(j + 1) * 128], ident[:, :])
                    nc.vector.tensor_copy(rs_cols[:, f * 4:f * 4 + 4],
                                          rsT_ps[:, f * 4:f * 4 + 4])

                    # K projection for this token chunk (both output halves)
                    for co in range(2):
                        cs = slice(co * 128, (co + 1) * 128)
                        ps = mmp.tile([128, 512], f32, tag="proj", name="proj")
                        for ci in range(2):
                            nc.tensor.matmul(ps[:, :], w_sb["k", ci][:, cs],
                                             xb[ci][:, fl], start=(ci == 0), stop=False)
                        nc.tensor.matmul(ps[:, :], wb_sb["k"][:, cs],
                                         musrt[:, fl], start=False, stop=True)
                        nc.vector.tensor_tensor(kT[co][:, fl], ps[:, :], rs_ball[:, fl], ALU.mult)

                    # Q projection (only first two chunks = this core's queries)
                    if f < 2:
                        for co in range(2):
                            cs = slice(co * 128, (co + 1) * 128)
                            ps = mmp.tile([128, 512], f32, tag="proj", name="proj")
                            for ci in range(2):
                                nc.tensor.matmul(ps[:, :], w_sb["q", ci][:, cs],
                                                 xb[ci][:, fl], start=(ci == 0), stop=False)
                            nc.tensor.matmul(ps[:, :], wb_sb["q"][:, cs],
                                             musrt[:, fl], start=False, stop=True)
                            for h in range(4):
                                rr = slice(h * 32, (h + 1) * 32)
                                nc.vector.tensor_tensor(qp[co][h][rr, fl], ps[rr, :],
                                                        rs_ball[rr, fl], ALU.mult)

                # V projection per 128-token chunk (tokens in partitions). The
                # 33rd "dummy" channel per head has zero weights and rank-1
                # bias = sqrt(var+eps), so after the *rstd evacuation it is
                # exactly the ones column (softmax denominator accumulator).
                for j in range(32):
                    jl = slice(j * 128, (j + 1) * 128)
                    ps = mmp.tile([128, NH * 33], f32, tag="proj", name="vproj")
                    for ci in range(2):
                        nc.tensor.matmul(ps[:, :], xb[ci][:, jl], w_sb["v", ci][:, :],
                                         start=(ci == 0), stop=False)
                    nc.tensor.matmul(ps[:, :], musrt[:, jl],
                                     wb_sb["v"][:, :], start=False, stop=True)
                    nc.scalar.mul(v_sb[:, j, :, :],
                                  ps[:, :].rearrange("p (h e) -> p h e", h=NH),
                                  rs_cols[:, j:j + 1])

            # ---- attention ----
            with tc.tile_pool(name="sps", bufs=1, space="PSUM") as sp, \
                 tc.tile_pool(name="avp", bufs=1, space="PSUM") as avp, \
                 tc.tile_pool(name="bcp", bufs=1, space="PSUM") as bcp, \
                 tc.tile_pool(name="pp", bufs=2) as ppool, \
                 tc.tile_pool(name="nrm", bufs=2) as nrm:
                with tc.tile_pool(name="mm2", bufs=1, space="PSUM") as mm2, \
                     tc.tile_pool(name="ot", bufs=4) as otp:
                    for f in range(2):
                        fl = slice(f * 512, (f + 1) * 512)
                        for hg in range(2):
                            av = [avp.tile([128, 512], f32, tag=f"av{pr}", name=f"av{pr}")
                                  for pr in range(2)]
                            for j in range(32):
                                jl = slice(j * 128, (j + 1) * 128)
                                ss = [sp.tile([128, 1024], f32, tag=f"s{i}", name=f"s{i}")
                                      for i in range(2)]
                                pt = [ppool.tile([128, 1024], bf16, tag=f"p{i}", name=f"p{i}")
                                      for i in range(2)]
                                for i in range(2):
                                    for t2 in range(2):
                                        h = i * 2 + t2
                                        nc.tensor.matmul(ss[i][:, t2 * 512:(t2 + 1) * 512],
                                                         kT[hg][:, jl], qp[hg][h][:, fl],
                                                         start=True, stop=True)
                                # pair 0: true exp on ScalarE; pair 1: Schraudolph on VectorE
                                nc.scalar.activation(pt[0][:, :], ss[0][:, :],
                                                     ACTF.Exp, scale=LN2 / 128.0)
                                nc.vector.tensor_scalar(pt[1][:, :].bitcast(i16), ss[1][:, :],
                                                        B16F, None, ALU.add)
                                for pr in range(2):
                                    for t2 in range(2):
                                        h = pr * 2 + t2
                                        nc.tensor.matmul(
                                            av[pr][t2 * 64:t2 * 64 + 33, :],
                                            v_sb[:, j, hg * 4 + h, :],
                                            pt[pr][:, t2 * 512:(t2 + 1) * 512],
                                            start=(j == 0), stop=(j == 31),
                                            tile_position=(0, t2 * 64))
                            # normalization: bit-trick reciprocal of the PSUM
                            # ones-row denominator, broadcast, multiply
                            for pr in range(2):
                                for t2 in range(2):
                                    db = nrm.tile([1, 512], bf16, tag="db", name="db")
                                    nc.vector.tensor_copy(
                                        db[:, :], av[pr][t2 * 64 + 32:t2 * 64 + 33, :])
                                    rcpb = nrm.tile([1, 512], bf16, tag="rb", name="rb")
                                    nc.vector.tensor_scalar(rcpb[:, :].bitcast(i16),
                                                            db[:, :].bitcast(i16),
                                                            -1.0, KRCP, ALU.mult, ALU.add)
                                    bcq = bcp.tile([32, 512], f32, tag="bcq", name="bcq")
                                    nc.tensor.matmul(bcq[:, :], ones_row[:, 0:32],
                                                     rcpb[:, :], start=True, stop=True)
                                    bcs = nrm.tile([32, 512], bf16, tag="bcs", name="bcs")
                                    nc.vector.tensor_copy(bcs[:, :], bcq[:, :])
                                    row0 = (pr * 2 + t2) * 32
                                    nc.vector.tensor_tensor(
                                        attnT[hg][row0:row0 + 32, fl],
                                        av[pr][t2 * 64:t2 * 64 + 32, :],
                                        bcs[:, :], ALU.mult)
                        # output projection + bias + residual for this f-chunk
                        # (overlaps the next attention group's matmuls)
                        for mo in range(2):
                            ms = slice(mo * 128, (mo + 1) * 128)
                            ps = mm2.tile([128, 512], f32, tag="o", name="o")
                            for ci in range(2):
                                nc.tensor.matmul(ps[:, :], w_sb["p", ci][:, ms],
                                                 attnT[ci][:, fl], start=(ci == 0), stop=(ci == 1))
                            ot = otp.tile([128, 512], f32, tag="ot", name="ot")
                            nc.vector.scalar_tensor_tensor(ot[:, :], ps[:, :], bp_sb[mo][:, :],
                                                           xf[mo][:, fl], ALU.add, ALU.add)
                            nc.sync.dma_start(out=od[ms, fl], in_=ot[:, :])

    nc.finalize()
    return nc


def kernel(x, ln_gamma, ln_beta, w_qkv, w_proj, b_proj):
    import ml_dtypes
    from concourse.bass_utils import run_bass_kernel_spmd

    if "nc" not in _CACHE:
        _CACHE["nc"] = _build()
    nc = _CACHE["nc"]

    bf = ml_dtypes.bfloat16
    x = np.asarray(x, np.float32)
    w_qkv = np.asarray(w_qkv, np.float32)
    gam = np.asarray(ln_gamma, np.float32)
    bet = np.asarray(ln_beta, np.float32)
    wq_, wk_, wv_ = w_qkv[0:C], w_qkv[C:2 * C], w_qkv[2 * C:3 * C]

    def prep(wmat, scale):
        wg = (scale * wmat * gam[None, :]).astype(bf)           # [o, c] gamma folded
        wT = np.ascontiguousarray(wg.T)                         # lhsT layout [in, out]
        sw = wg.astype(np.float32).sum(1)                       # rowsum of device weights
        bias = scale * (wmat @ bet)
        return wT, np.ascontiguousarray(np.stack([bias, -sw]).astype(bf))

    wqT, wbq_h = prep(wq_, A_SCALE)
    wkT, wbk_h = prep(wk_, 1.0)
    # V extended with a zero-weight dummy channel per head whose rank-1 bias
    # is 1 against the srt row (becomes the softmax-denominator ones column).
    wvg = (wv_ * gam[None, :]).astype(bf)
    wv_ext = np.zeros((NH * 33, C), bf)
    wbv_h = np.zeros((2, NH * 33), np.float32)
    for h in range(NH):
        wv_ext[h * 33:h * 33 + 32] = wvg[h * 32:(h + 1) * 32]
        wbv_h[1, h * 33:h * 33 + 32] = -wvg[h * 32:(h + 1) * 32].astype(np.float32).sum(1)
        wbv_h[0, h * 33:h * 33 + 32] = (wv_ @ bet)[h * 32:(h + 1) * 32]
        wbv_h[0, h * 33 + 32] = 1.0
    wvT = np.ascontiguousarray(wv_ext.T)
    wbv_h = wbv_h.astype(bf)
    wpT = np.ascontiguousarray(np.asarray(w_proj, np.float32).T.astype(bf))
    bp = np.asarray(b_proj, np.float32).reshape(C, 1)

    xfull = x.reshape(B, C, N)
    in_maps = []
    for core in range(8):
        b, qc = core // 4, core % 4
        xr = np.roll(xfull[b], -qc * NQ, axis=1)
        in_maps.append({
            "xb": np.ascontiguousarray(xr.astype(bf)),
            "xf": np.ascontiguousarray(xr[:, :NQ]),
            "wqT": wqT, "wkT": wkT, "wvT": wvT, "wpT": wpT,
            "wbq": wbq_h, "wbk": wbk_h, "wbv": wbv_h, "bp": bp,
        })

    res = run_bass_kernel_spmd(nc, in_maps, core_ids=list(range(8)),
                               trace=_PROFILE)
    if _PROFILE:
        _CACHE["exec_time_ns"] = res.exec_time_ns
    out = np.empty((B, C, N), np.float32)
    for core in range(8):
        b, qc = core // 4, core % 4
        out[b][:, qc * NQ:(qc + 1) * NQ] = res.results[core]["out"]
    return out.reshape(B, C, H, W)


# revision 44
# speedup vs baseline: 1.0422x; 1.0086x over previous
"""Trainium2 Bass kernel for nn_Attention (B=2, C=256, H=W=64, 8 heads).

Sharding: 8 cores = 2 batches x 4 query-chunks (1024 queries each), no
collectives. Each core gets its batch's full x (bf16) with token columns
rolled so its own query chunk sits at columns 0:1024 (attention is
permutation-invariant over keys); it computes LN + projections + attention
for its queries and writes a [256, 1024] slice of the output.

Key structure (v2 — rebuilt for PE throughput; 703us -> 470us):
- x ships as bf16 (attention path + LN stats) plus an f32 query-slice for
  the exact residual.
- LN is folded into the projections: gamma is pre-multiplied into the
  weights on the host; the per-token mean and sqrt(var+eps) enter as two
  rank-1 fixup matmuls accumulated into each projection's PSUM
  (-rowsum(W') x mu and (W@beta) x srt), followed by a *rstd multiply at
  PSUM evacuation. No normalized-x tensor is ever materialized. rstd and
  sqrt come from bf16 exponent bit-tricks on VectorE (the custom-DVE fast
  reciprocal is broken on this stack, and mixing Ln into ScalarE thrashes
  the ACT table against Exp at 2.7us/reload).
- S^T matmuls are full-array (unmasked): the stationary is the whole
  4-head K chunk [128x128]; per-head Q lives in zero-padded [128, NQ]
  tiles so each 512-query matmul contracts over all 128 channel rows but
  only the head's 32 rows are nonzero. This keeps the PE HAM clock-gate
  at 2.4 GHz (masked tile_position matmuls don't count as PE activity —
  the whole attention phase otherwise runs at the cold 1.2 GHz clock).
- exp splits per key-chunk between ScalarE (true exp on head-pair 0) and
  VectorE (Schraudolph bit-trick on head-pair 1): Wq is pre-scaled so
  PSUM = 128*log2(e)*logit; adding a magic bias and converting f32->int16
  yields the bf16 bit pattern of 2^y (~3% max rel err, harmless next to
  the residual). Both pairs process [128,1024] two-bank PSUM tiles.
- P^T V via stationary-V matmuls whose 33rd per-head column is a dummy
  output channel (zero weights, rank-1 bias = srt, so after the *rstd
  evacuation it is exactly 1): the AV accumulation emits the softmax
  denominators for free. Normalization is a bf16 bit-trick reciprocal
  broadcast down 32 rows through a tiny bf16 matmul.
- Per-f output projection (+bias +f32 residual in one fused
  scalar_tensor_tensor) overlaps the next attention group.
"""

import numpy as np

B, C, H, W = 2, 256, 64, 64
N = H * W            # 4096 tokens
NH, HD = 8, 32       # heads, head_dim
NQ = N // 4          # queries per core
LN_EPS = 1e-5
LOG2E = 1.4426950408889634
LN2 = 0.6931471805599453
ATTN_SCALE = HD ** -0.5
A_SCALE = 128.0 * LOG2E * ATTN_SCALE   # folded into Wq on host
B16F = 16256.0 - 5.6                   # Schraudolph bias (calibrated)
KRSQ = 24375.25                        # bf16 bit-trick rsqrt bias (<=3.7% rel)
KSQ = 8123.0                           # bf16 bit-trick sqrt bias (<=3.9% rel)
KRCP = 32498.75                        # bf16 bit-trick recip bias (<=5.3% rel)

_PROFILE = False
_CACHE = {}


def _build():
    from concourse import bacc
    from concourse import mybir
    import concourse.tile as tile

    f32 = mybir.dt.float32
    f32r = mybir.dt.float32r
    bf16 = mybir.dt.bfloat16
    i16 = mybir.dt.int16
    ALU = mybir.AluOpType
    ACTF = mybir.ActivationFunctionType

    nc = bacc.Bacc("TRN2", target_bir_lowering=False)
    xbd = nc.dram_tensor("xb", [C, N], bf16, kind="ExternalInput")
    xfd = nc.dram_tensor("xf", [C, NQ], f32, kind="ExternalInput")
    wq = nc.dram_tensor("wqT", [C, C], bf16, kind="ExternalInput")  # gamma+A_SCALE folded
    wk = nc.dram_tensor("wkT", [C, C], bf16, kind="ExternalInput")
    wv = nc.dram_tensor("wvT", [C, NH * 33], bf16, kind="ExternalInput")
    wp = nc.dram_tensor("wpT", [C, C], bf16, kind="ExternalInput")
    # rank-1 LN fixup pairs: row0 = W@beta (pairs sqrt(var+eps)), row1 =
    # -rowsum(W') (pairs mu) — one contract-2 fixup matmul per projection chunk
    wbq = nc.dram_tensor("wbq", [2, C], bf16, kind="ExternalInput")
    wbk = nc.dram_tensor("wbk", [2, C], bf16, kind="ExternalInput")
    wbv = nc.dram_tensor("wbv", [2, NH * 33], bf16, kind="ExternalInput")
    bpd = nc.dram_tensor("bp", [C, 1], f32, kind="ExternalInput")
    od = nc.dram_tensor("out", [C, NQ], f32, kind="ExternalOutput")

    with tile.TileContext(nc) as tc:
        with tc.tile_pool(name="big", bufs=1) as big, \
             tc.tile_pool(name="sml", bufs=2) as sml:

            # ---- load inputs ----
            xb = [big.tile([128, N], bf16, tag=f"xb{c}", name=f"xb{c}") for c in range(2)]
            for q4 in range(4):
                qs = slice(q4 * 1024, (q4 + 1) * 1024)
                for c in range(2):
                    nc.sync.dma_start(out=xb[c][:, qs], in_=xbd[c * 128:(c + 1) * 128, qs])
            xf = [big.tile([128, NQ], f32, tag=f"xf{c}", name=f"xf{c}") for c in range(2)]
            for c in range(2):
                nc.sync.dma_start(out=xf[c][:, :], in_=xfd[c * 128:(c + 1) * 128, :])
            w_sb = {}
            for name, t, nout in (("q", wq, C), ("k", wk, C),
                                  ("v", wv, NH * 33), ("p", wp, C)):
                for c in range(2):
                    s = big.tile([128, nout], bf16, tag=f"w{name}{c}", name=f"w{name}{c}")
                    nc.sync.dma_start(out=s[:, :], in_=t[c * 128:(c + 1) * 128, :])
                    w_sb[name, c] = s
            wb_sb = {}
            for name, t, nout in (("q", wbq, C), ("k", wbk, C), ("v", wbv, NH * 33)):
                s = big.tile([2, nout], bf16, tag=f"wb{name}", name=f"wb{name}")
                nc.sync.dma_start(out=s[:, :], in_=t[:, :])
                wb_sb[name] = s
            bp_sb = [big.tile([128, 1], f32, tag=f"bp{c}", name=f"bp{c}") for c in range(2)]
            for c in range(2):
                nc.sync.dma_start(out=bp_sb[c][:, :], in_=bpd[c * 128:(c + 1) * 128, :])

            onesC = big.tile([128, 1], bf16, tag="onesC", name="onesC")
            nc.vector.memset(onesC[:, :], 1.0 / C)
            ones_row = big.tile([1, 128], bf16, tag="onesr", name="onesr")
            nc.vector.memset(ones_row[:, :], 1.0)
            ident = big.tile([1, 1], f32, tag="ident", name="ident")
            nc.vector.memset(ident[:, :], 1.0)

            rs_cols = big.tile([128, 32], f32, tag="rscols", name="rscols")  # rstd, col layout

            kT = [big.tile([128, N], bf16, tag=f"kT{c}", name=f"kT{c}") for c in range(2)]
            qp = [[big.tile([128, NQ], bf16, tag=f"qp{hg}{h}", name=f"qp{hg}{h}")
                   for h in range(4)] for hg in range(2)]
            for hg in range(2):
                for h in range(4):
                    nc.gpsimd.memset(qp[hg][h][:, :], 0.0)
            v_sb = big.tile([128, 32, NH, 33], bf16, tag="v", name="v")
            attnT = [big.tile([128, NQ], bf16, tag=f"at{c}", name=f"at{c}") for c in range(2)]

            # ---- LN stats + projections ----
            with tc.tile_pool(name="lnsb", bufs=1) as lnsb, \
                 tc.tile_pool(name="lnp", bufs=1, space="PSUM") as lnp, \
                 tc.tile_pool(name="wrm", bufs=2, space="PSUM") as wrm, \
                 tc.tile_pool(name="mm", bufs=2, space="PSUM") as mmp:
                # ~4.4us of back-to-back full-array matmuls on the (tiny,
                # early-arriving) weight tiles: releases the HAM clock-gate to
                # 2.4 GHz during the x DMA wait, so the stats/fixup phase (all
                # masked matmuls that don't count as PE activity) runs warm.
                for w in range(20):
                    wps = wrm.tile([128, NH * 33], f32, tag="w", name="w")
                    nc.tensor.matmul(wps[:, :], w_sb["p", w % 2][:, 0:128],
                                     w_sb["v", w % 2][:, :], start=True, stop=True)
                mu_row = lnsb.tile([1, N], bf16, tag="murow", name="murow")
                musrt = lnsb.tile([2, N], bf16, tag="musrt", name="musrt")
                rs_row = lnsb.tile([1, N], f32, tag="rsrow", name="rsrow")
                rs_bf = lnsb.tile([1, N], bf16, tag="rsbf", name="rsbf")
                rs_ball = lnsb.tile([128, N], f32, tag="rsball", name="rsball")
                xsq = [lnsb.tile([128, N], bf16, tag=f"xsq{c}", name=f"xsq{c}") for c in range(2)]
                nc.vector.tensor_tensor(xsq[0][:, :], xb[0][:, :], xb[0][:, :], ALU.mult)
                nc.vector.tensor_tensor(xsq[1][:, :], xb[1][:, :], xb[1][:, :], ALU.mult)
                rsT_ps = lnp.tile([128, 32], f32, tag="rsT", name="rsT")
                for f in range(8):
                    fl = slice(f * 512, (f + 1) * 512)
                    mps = lnp.tile([1, 512], f32, tag="mps", name="mps")
                    nc.tensor.matmul(mps[:, :], onesC[:, :], xb[0][:, fl], start=True, stop=False)
                    nc.tensor.matmul(mps[:, :], onesC[:, :], xb[1][:, fl], start=False, stop=True)
                    sps = lnp.tile([1, 512], f32, tag="sps", name="sps")
                    nc.tensor.matmul(sps[:, :], onesC[:, :], xsq[0][:, fl], start=True, stop=False)
                    nc.tensor.matmul(sps[:, :], onesC[:, :], xsq[1][:, fl], start=False, stop=True)
                    # mu row (SBUF, bf16) + vare = (msq + eps) - mu^2
                    nc.scalar.copy(mu_row[0:1, fl], mps[:, :])
                    mu2 = sml.tile([1, 512], f32, tag="mu2", name="mu2")
                    nc.vector.tensor_tensor(mu2[:, :], mu_row[0:1, fl], mu_row[0:1, fl], ALU.mult)
                    vare = sml.tile([1, 512], f32, tag="vare", name="vare")
                    nc.vector.scalar_tensor_tensor(vare[:, :], sps[:, :], LN_EPS, mu2[:, :],
                                                   ALU.add, ALU.subtract)
                    # rstd / sqrt via bf16 exponent bit tricks on VectorE
                    # (keeps ScalarE's ACT table pinned to the Exp set; the
                    # custom-DVE fast reciprocal is broken on this stack)
                    vb = sml.tile([1, 512], bf16, tag="vb", name="vb")
                    nc.vector.tensor_copy(vb[:, :], vare[:, :])
                    nc.vector.tensor_scalar(rs_bf[0:1, fl].bitcast(i16),
                                            vb[:, :].bitcast(i16),
                                            -0.5, KRSQ, ALU.mult, ALU.add)
                    nc.vector.tensor_scalar(musrt[0:1, fl].bitcast(i16),
                                            vb[:, :].bitcast(i16),
                                            0.5, KSQ, ALU.mult, ALU.add)
                    nc.scalar.copy(rs_row[0:1, fl], rs_bf[0:1, fl])
                    # mu into musrt row1: engines cannot write partition 1,
                    # but an SBUF->SBUF DMA can
                    nc.sync.dma_start(out=musrt[1:2, fl], in_=mu_row[0:1, fl])
                    # rs broadcast down 128 rows (bf16 matmul) -> SBUF
                    rsb_ps = lnp.tile([128, 512], f32, tag="rsb", name="rsb")
                    nc.tensor.matmul(rsb_ps[:, :], ones_row[:, :], rs_bf[0:1, fl],
                                     start=True, stop=True)
                    nc.vector.tensor_copy(rs_ball[:, fl], rsb_ps[:, :])
                    # rstd row -> column layout (PE transposes, 4 chunks of 128)
                    for t in range(4):
                        j = f * 4 + t
                        nc.tensor.transpose(rsT_ps[:, j:j + 1],
                                            rs_row[0:1, j * 128:(j + 1) * 128], ident[:, :])
                    nc.vector.tensor_copy(rs_cols[:, f * 4:f * 4 + 4],
                                          rsT_ps[:, f * 4:f * 4 + 4])

                    # K projection for this token chunk (both output halves)
                    for co in range(2):
                        cs = slice(co * 128, (co + 1) * 128)
                        ps = mmp.tile([128, 512], f32, tag="proj", name="proj")
                        for ci in range(2):
                            nc.tensor.matmul(ps[:, :], w_sb["k", ci][:, cs],
                                             xb[ci][:, fl], start=(ci == 0), stop=False)
                        nc.tensor.matmul(ps[:, :], wb_sb["k"][:, cs],
                                         musrt[:, fl], start=False, stop=True)
                        nc.vector.tensor_tensor(kT[co][:, fl], ps[:, :], rs_ball[:, fl], ALU.mult)

                    # Q projection (only first two chunks = this core's queries)
                    if f < 2:
                        for co in range(2):
                            cs = slice(co * 128, (co + 1) * 128)
                            ps = mmp.tile([128, 512], f32, tag="proj", name="proj")
                            for ci in range(2):
                                nc.tensor.matmul(ps[:, :], w_sb["q", ci][:, cs],
                                                 xb[ci][:, fl], start=(ci == 0), stop=False)
                            nc.tensor.matmul(ps[:, :], wb_sb["q"][:, cs],
                                             musrt[:, fl], start=False, stop=True)
                            for h in range(4):
                                rr = slice(h * 32, (h + 1) * 32)
                                nc.vector.tensor_tensor(qp[co][h][rr, fl], ps[rr, :],
                                                        rs_ball[rr, fl], ALU.mult)

                # V projection per 128-token chunk (tokens in partitions). The
                # 33rd "dummy" channel per head has zero weights and rank-1
                # bias = sqrt(var+eps), so after the *rstd evacuation it is
                # exactly the ones column (softmax denominator accumulator).
                for j in range(32):
                    jl = slice(j * 128, (j + 1) * 128)
                    ps = mmp.tile([128, NH * 33], f32, tag="proj", name="vproj")
                    for ci in range(2):
                        nc.tensor.matmul(ps[:, :], xb[ci][:, jl], w_sb["v", ci][:, :],
                                         start=(ci == 0), stop=False)
                    nc.tensor.matmul(ps[:, :], musrt[:, jl],
                                     wb_sb["v"][:, :], start=False, stop=True)
                    nc.scalar.mul(v_sb[:, j, :, :],
                                  ps[:, :].rearrange("p (h e) -> p h e", h=NH),
                                  rs_cols[:, j:j + 1])

            # ---- attention ----
            with tc.tile_pool(name="sps", bufs=1, space="PSUM") as sp, \
                 tc.tile_pool(name="avp", bufs=1, space="PSUM") as avp, \
                 tc.tile_pool(name="bcp", bufs=1, space="PSUM") as bcp, \
                 tc.tile_pool(name="pp", bufs=2) as ppool, \
                 tc.tile_pool(name="nrm", bufs=2) as nrm:
                with tc.tile_pool(name="mm2", bufs=1, space="PSUM") as mm2, \
                     tc.tile_pool(name="ot", bufs=4) as otp:
                    for f in range(2):
                        fl = slice(f * 512, (f + 1) * 512)
                        for hg in range(2):
                            av = [avp.tile([128, 512], f32, tag=f"av{pr}", name=f"av{pr}")
                                  for pr in range(2)]
                            for j in range(32):
                                jl = slice(j * 128, (j + 1) * 128)
                                ss = [sp.tile([128, 1024], f32, tag=f"s{i}", name=f"s{i}")
                                      for i in range(2)]
                                pt = [ppool.tile([128, 1024], bf16, tag=f"p{i}", name=f"p{i}")
                                      for i in range(2)]
                                for i in range(2):
                                    for t2 in range(2):
                                        h = i * 2 + t2
                                        nc.tensor.matmul(ss[i][:, t2 * 512:(t2 + 1) * 512],
                                                         kT[hg][:, jl], qp[hg][h][:, fl],
                                                         start=True, stop=True)
                                # pair 0: true exp on ScalarE; pair 1: Schraudolph on VectorE
                                nc.scalar.activation(pt[0][:, :], ss[0][:, :],
                                                     ACTF.Exp, scale=LN2 / 128.0)
                                nc.vector.tensor_scalar(pt[1][:, :].bitcast(i16), ss[1][:, :],
                                                        B16F, None, ALU.add)
                                for pr in range(2):
                                    for t2 in range(2):
                                        h = pr * 2 + t2
                                        nc.tensor.matmul(
                                            av[pr][t2 * 64:t2 * 64 + 33, :],
                                            v_sb[:, j, hg * 4 + h, :],
                                            pt[pr][:, t2 * 512:(t2 + 1) * 512],
                                            start=(j == 0), stop=(j == 31),
                                            tile_position=(0, t2 * 64))
                            # normalization: bit-trick reciprocal of the PSUM
                            # ones-row denominator, broadcast, multiply
                            for pr in range(2):
                                for t2 in range(2):
                                    db = nrm.tile([1, 512], bf16, tag="db", name="db")
                                    nc.scalar.copy(
                                        db[:, :], av[pr][t2 * 64 + 32:t2 * 64 + 33, :])
                                    rcpb = nrm.tile([1, 512], bf16, tag="rb", name="rb")
                                    nc.vector.tensor_scalar(rcpb[:, :].bitcast(i16),
                                                            db[:, :].bitcast(i16),
                                                            -1.0, KRCP, ALU.mult, ALU.add)
                                    bcq = bcp.tile([32, 512], f32, tag="bcq", name="bcq")
                                    nc.tensor.matmul(bcq[:, :], ones_row[:, 0:32],
                                                     rcpb[:, :], start=True, stop=True)
                                    bcs = nrm.tile([32, 512], bf16, tag="bcs", name="bcs")
                                    nc.scalar.copy(bcs[:, :], bcq[:, :])
                                    row0 = (pr * 2 + t2) * 32
                                    nc.vector.tensor_tensor(
                                        attnT[hg][row0:row0 + 32, fl],
                                        av[pr][t2 * 64:t2 * 64 + 32, :],
                                        bcs[:, :], ALU.mult)
                        # output projection + bias + residual for this f-chunk
                        # (overlaps the next attention group's matmuls)
                        for mo in range(2):
                            ms = slice(mo * 128, (mo + 1) * 128)
                            ps = mm2.tile([128, 512], f32, tag="o", name="o")
                            for ci in range(2):
                                nc.tensor.matmul(ps[:, :], w_sb["p", ci][:, ms],
                                                 attnT[ci][:, fl], start=(ci == 0), stop=(ci == 1))
                            ot = otp.tile([128, 512], f32, tag="ot", name="ot")
                            nc.vector.scalar_tensor_tensor(ot[:, :], ps[:, :], bp_sb[mo][:, :],
                                                           xf[mo][:, fl], ALU.add, ALU.add)
                            nc.sync.dma_start(out=od[ms, fl], in_=ot[:, :])

    nc.finalize()
    return nc


def kernel(x, ln_gamma, ln_beta, w_qkv, w_proj, b_proj):
    import ml_dtypes
    from concourse.bass_utils import run_bass_kernel_spmd

    if "nc" not in _CACHE:
        _CACHE["nc"] = _build()
    nc = _CACHE["nc"]

    bf = ml_dtypes.bfloat16
    x = np.asarray(x, np.float32)
    w_qkv = np.asarray(w_qkv, np.float32)
    gam = np.asarray(ln_gamma, np.float32)
    bet = np.asarray(ln_beta, np.float32)
    wq_, wk_, wv_ = w_qkv[0:C], w_qkv[C:2 * C], w_qkv[2 * C:3 * C]

    def prep(wmat, scale):
        wg = (scale * wmat * gam[None, :]).astype(bf)           # [o, c] gamma folded
        wT = np.ascontiguousarray(wg.T)                         # lhsT layout [in, out]
        sw = wg.astype(np.float32).sum(1)                       # rowsum of device weights
        bias = scale * (wmat @ bet)
        return wT, np.ascontiguousarray(np.stack([bias, -sw]).astype(bf))

    wqT, wbq_h = prep(wq_, A_SCALE)
    wkT, wbk_h = prep(wk_, 1.0)
    # V extended with a zero-weight dummy channel per head whose rank-1 bias
    # is 1 against the srt row (becomes the softmax-denominator ones column).
    wvg = (wv_ * gam[None, :]).astype(bf)
    wv_ext = np.zeros((NH * 33, C), bf)
    wbv_h = np.zeros((2, NH * 33), np.float32)
    for h in range(NH):
        wv_ext[h * 33:h * 33 + 32] = wvg[h * 32:(h + 1) * 32]
        wbv_h[1, h * 33:h * 33 + 32] = -wvg[h * 32:(h + 1) * 32].astype(np.float32).sum(1)
        wbv_h[0, h * 33:h * 33 + 32] = (wv_ @ bet)[h * 32:(h + 1) * 32]
        wbv_h[0, h * 33 + 32] = 1.0
    wvT = np.ascontiguousarray(wv_ext.T)
    wbv_h = wbv_h.astype(bf)
    wpT = np.ascontiguousarray(np.asarray(w_proj, np.float32).T.astype(bf))
    bp = np.asarray(b_proj, np.float32).reshape(C, 1)

    xfull = x.reshape(B, C, N)
    in_maps = []
    for core in range(8):
        b, qc = core // 4, core % 4
        xr = np.roll(xfull[b], -qc * NQ, axis=1)
        in_maps.append({
            "xb": np.ascontiguousarray(xr.astype(bf)),
            "xf": np.ascontiguousarray(xr[:, :NQ]),
            "wqT": wqT, "wkT": wkT, "wvT": wvT, "wpT": wpT,
            "wbq": wbq_h, "wbk": wbk_h, "wbv": wbv_h, "bp": bp,
        })

    res = run_bass_kernel_spmd(nc, in_maps, core_ids=list(range(8)),
                               trace=_PROFILE)
    if _PROFILE:
        _CACHE["exec_time_ns"] = res.exec_time_ns
    out = np.empty((B, C, N), np.float32)
    for core in range(8):
        b, qc = core // 4, core % 4
        out[b][:, qc * NQ:(qc + 1) * NQ] = res.results[core]["out"]
    return out.reshape(B, C, H, W)
